# revision 10
# baseline (speedup 1.0000x reference)
"""Block-diagonal linear layer on 8 trn2 NeuronCores.

Reference op:  out = x @ tanh(W * mask).T
  x    [8192, 4096] f32
  W    [4096, 4096] f32, random inside 8 diagonal 512x512 blocks, 0 outside
  mask [4096, 4096] bool, True exactly on the 8 diagonal 512x512 blocks

tanh(0) == 0, so eff = tanh(W*mask) is block-diagonal: out[:, blk_k] depends
only on x[:, blk_k] and W[blk_k, blk_k].  Sharding: block k -> core k
(expert-style), zero inter-core communication.

Per-core device program (SPMD, same NEFF on all 8 cores):
  inputs   xt [512, 8192] f16  =  x[:, blk].T     (host transpose + f16 cast)
           wt [512, 512]  f32  =  W[blk, blk].T
  output   ot [512, 8192] f16  =  tanh(W_blk) @ x_blk.T   (i.e. out[:, blk].T)

The transposed staging keeps every DMA natural-stride (contraction dim i on
SBUF partitions for both matmul operands, no on-chip transposes).  f16 keeps
the PE at 1 cycle/row (f32 matmul is 4x slower) and halves DMA traffic;
with K=512 dots the f16 rounding gives ~1e-4 relative error.
"""

from contextlib import ExitStack

import numpy as np

BLOCK = 512
NBLOCKS = 8
BATCH = 8192
N = BLOCK * NBLOCKS

KI = BLOCK // 128  # 4 contraction chunks of 128 (SBUF partition dim)
OT = BLOCK // 128  # 4 output-row tiles of 128
BT = 512           # batch tile (one PSUM bank of f32)
NB = BATCH // BT   # 16 batch tiles

_CACHED = {}


def _build_program():
    import concourse.bacc as bacc
    import concourse.bass as bass
    import concourse.mybir as mybir
    import concourse.tile as tile

    f16 = mybir.dt.float16
    f32 = mybir.dt.float32

    nc = bacc.Bacc(
        "TRN2",
        target_bir_lowering=False,
        debug=False,
        enable_asserts=False,
        num_devices=NBLOCKS,
    )

    xt = nc.dram_tensor("xt", [BLOCK, BATCH], f16, kind="ExternalInput").ap()
    wt = nc.dram_tensor("wt", [BLOCK, BLOCK], f16, kind="ExternalInput").ap()
    ot = nc.dram_tensor("ot", [BLOCK, BATCH], f16, kind="ExternalOutput").ap()

    QUAD = 2048              # batch columns per x-load / out-store DMA
    NQ = BATCH // QUAD       # 4 quads
    HT = QUAD // BT          # 4 batch tiles per quad

    # i-major views: partition dim = 128 contraction lanes, then chunk, then col
    xtv = xt.rearrange("(c p) b -> p c b", p=128)
    wtv = wt.rearrange("(c p) o -> p c o", p=128)

    with tile.TileContext(nc) as tc, ExitStack() as ctx:
        wpool = ctx.enter_context(tc.tile_pool(name="w", bufs=1))
        xpool = ctx.enter_context(tc.tile_pool(name="x", bufs=4))
        opool = ctx.enter_context(tc.tile_pool(name="o", bufs=2))
        pspool = ctx.enter_context(tc.tile_pool(name="ps", bufs=2, space="PSUM"))

        # PE warmup: dummy matmuls on a memset tile so the HAM clock-gate
        # opens (~3.4us of sustained PE activity) and stays open while the
        # real loads land.  Rotating over all 4 psum tags keeps them dense.
        xwarm = wpool.tile([128, BT], f16, tag="warm", name="xwarm")
        nc.gpsimd.memset(xwarm[:], 0.0)
        for r in range(25):
            pw = pspool.tile([128, BT], f32, tag=f"pb{r % 4}", name=f"warm{r}")
            nc.tensor.matmul(pw[:], xwarm[:, :128], xwarm[:], start=True, stop=True)

        # eff^T = tanh(W^T) staged as [i_lane, i_chunk, o] f16.  W (f16) and
        # the first two h-pieces of quad 0 go over SWDGE (gpsimd is alive
        # within ~0.5us, long before the HWDGE rings spin up); tanh is
        # chunked so the first matmul group can start after chunk 0.
        wti = wpool.tile([128, KI, BLOCK], f16, tag="w", name="wti")
        nc.gpsimd.dma_start(wti[:], wtv[:])
        eff = wpool.tile([128, KI, BLOCK], f16, tag="e", name="eff")
        for i in range(KI):
            nc.scalar.activation(
                eff[:, i, :], wti[:, i, :], mybir.ActivationFunctionType.Tanh
            )

        for q in range(NQ):
            xq = xpool.tile([128, KI, QUAD], f16, tag="x", name=f"xq{q}")
            if q == 0:
                # h-granular pieces so the first matmul group starts after
                # ~512KB instead of the whole 2MB quad; first two via SWDGE
                for h in range(HT):
                    eng = nc.gpsimd if h < 2 else nc.sync
                    eng.dma_start(
                        xq[:, :, BT * h : BT * (h + 1)],
                        xtv[:, :, BT * h : BT * (h + 1)],
                    )
            else:
                nc.sync.dma_start(xq[:], xtv[:, :, QUAD * q : QUAD * (q + 1)])

            for hh in range(HT // 2):
                stg = [
                    opool.tile(
                        [128, 2 * BT], f16, tag=f"o{o}h{hh}", name=f"st{o}_{q}_{hh}"
                    )
                    for o in range(OT)
                ]
                if q == 0:
                    # h-major: consume the arriving pieces in order
                    for hl in range(2):
                        h = 2 * hh + hl
                        for o in range(OT):
                            ps = pspool.tile(
                                [128, BT], f32, tag=f"pb{o}", name=f"ps{o}_{q}_{h}"
                            )
                            for i in range(KI):
                                nc.tensor.matmul(
                                    ps[:],
                                    eff[:, i, 128 * o : 128 * (o + 1)],
                                    xq[:, i, BT * h : BT * (h + 1)],
                                    start=(i == 0),
                                    stop=(i == KI - 1),
                                )
                            dst = stg[o][:, BT * hl : BT * (hl + 1)]
                            if o % 2 == 0:
                                nc.vector.tensor_copy(dst, ps[:])
                            else:
                                nc.scalar.copy(dst, ps[:])
                else:
                    # weight-reuse order: for each (o, i) the 4 consecutive
                    # matmuls share the stationary operand (one LDWEIGHTS),
                    # accumulating into 2 interleaved h-banks per half
                    for o in range(OT):
                        pss = [
                            pspool.tile(
                                [128, BT], f32, tag=f"pb{2 * (o % 2) + hl}",
                                name=f"ps{o}_{q}_{2 * hh + hl}",
                            )
                            for hl in range(2)
                        ]
                        for i in range(KI):
                            for hl in range(2):
                                h = 2 * hh + hl
                                nc.tensor.matmul(
                                    pss[hl][:],
                                    eff[:, i, 128 * o : 128 * (o + 1)],
                                    xq[:, i, BT * h : BT * (h + 1)],
                                    start=(i == 0),
                                    stop=(i == KI - 1),
                                )
                        for hl in range(2):
                            dst = stg[o][:, BT * hl : BT * (hl + 1)]
                            if o % 2 == 0:
                                nc.vector.tensor_copy(dst, pss[hl][:])
                            else:
                                nc.scalar.copy(dst, pss[hl][:])
                for o in range(OT):
                    # stores on the ACT HWDGE ring, disjoint from the load ring
                    nc.scalar.dma_start(
                        ot[
                            128 * o : 128 * (o + 1),
                            QUAD * q + 2 * BT * hh : QUAD * q + 2 * BT * (hh + 1),
                        ],
                        stg[o][:],
                    )

    nc.compile()
    return nc


def get_program():
    if "nc" not in _CACHED:
        _CACHED["nc"] = _build_program()
    return _CACHED["nc"]


def make_in_maps(x: np.ndarray, W: np.ndarray):
    xT16 = x.T.astype(np.float16)  # [N, BATCH] C-contiguous
    in_maps = []
    for k in range(NBLOCKS):
        sl = slice(BLOCK * k, BLOCK * (k + 1))
        in_maps.append(
            {
                "xt": np.ascontiguousarray(xT16[sl, :]),
                "wt": np.ascontiguousarray(W[sl, sl].T.astype(np.float16)),
            }
        )
    return in_maps


def assemble_output(results) -> np.ndarray:
    out = np.empty((BATCH, N), np.float32)
    for k in range(NBLOCKS):
        out[:, BLOCK * k : BLOCK * (k + 1)] = results[k]["ot"].T.astype(np.float32)
    return out


def kernel(x: np.ndarray, W: np.ndarray, mask: np.ndarray) -> np.ndarray:
    # mask is exactly the block-diagonal pattern (all-True inside each
    # diagonal 512 block); W is already zero off-block, so tanh(W*mask)
    # restricted to block k is tanh(W[blk_k, blk_k]).
    from concourse.bass_utils import run_bass_kernel_spmd

    nc = get_program()
    in_maps = make_in_maps(x, W)
    res = run_bass_kernel_spmd(nc, in_maps, list(range(NBLOCKS)))
    return assemble_output(res.results)


# revision 12
# speedup vs baseline: 1.0579x; 1.0579x over previous
"""Block-diagonal linear layer on 8 trn2 NeuronCores.

Reference op:  out = x @ tanh(W * mask).T
  x    [8192, 4096] f32
  W    [4096, 4096] f32, random inside 8 diagonal 512x512 blocks, 0 outside
  mask [4096, 4096] bool, True exactly on the 8 diagonal 512x512 blocks

tanh(0) == 0, so eff = tanh(W*mask) is block-diagonal: out[:, blk_k] depends
only on x[:, blk_k] and W[blk_k, blk_k].  Sharding: block k -> core k
(expert-style), zero inter-core communication.

Per-core device program (SPMD, same NEFF on all 8 cores):
  inputs   xt [512, 8192] f16  =  x[:, blk].T     (host transpose + f16 cast)
           wt [512, 512]  f32  =  W[blk, blk].T
  output   ot [512, 8192] f16  =  tanh(W_blk) @ x_blk.T   (i.e. out[:, blk].T)

The transposed staging keeps every DMA natural-stride (contraction dim i on
SBUF partitions for both matmul operands, no on-chip transposes).  f16 keeps
the PE at 1 cycle/row (f32 matmul is 4x slower) and halves DMA traffic;
with K=512 dots the f16 rounding gives ~1e-4 relative error.
"""

from contextlib import ExitStack

import numpy as np

BLOCK = 512
NBLOCKS = 8
BATCH = 8192
N = BLOCK * NBLOCKS

KI = BLOCK // 128  # 4 contraction chunks of 128 (SBUF partition dim)
OT = BLOCK // 128  # 4 output-row tiles of 128
BT = 512           # batch tile (one PSUM bank of f32)
NB = BATCH // BT   # 16 batch tiles

_CACHED = {}


def _build_program():
    import concourse.bacc as bacc
    import concourse.bass as bass
    import concourse.mybir as mybir
    import concourse.tile as tile

    f16 = mybir.dt.float16
    f32 = mybir.dt.float32

    nc = bacc.Bacc(
        "TRN2",
        target_bir_lowering=False,
        debug=False,
        enable_asserts=False,
        num_devices=NBLOCKS,
    )

    xt = nc.dram_tensor("xt", [BLOCK, BATCH], f16, kind="ExternalInput").ap()
    wt = nc.dram_tensor("wt", [BLOCK, BLOCK], f16, kind="ExternalInput").ap()
    ot = nc.dram_tensor("ot", [BLOCK, BATCH], f16, kind="ExternalOutput").ap()

    QUAD = 2048              # batch columns per x-load / out-store DMA
    NQ = BATCH // QUAD       # 4 quads
    HT = QUAD // BT          # 4 batch tiles per quad

    # i-major views: partition dim = 128 contraction lanes, then chunk, then col
    xtv = xt.rearrange("(c p) b -> p c b", p=128)
    wtv = wt.rearrange("(c p) o -> p c o", p=128)

    with tile.TileContext(nc) as tc, ExitStack() as ctx:
        wpool = ctx.enter_context(tc.tile_pool(name="w", bufs=1))
        xpool = ctx.enter_context(tc.tile_pool(name="x", bufs=4))
        opool = ctx.enter_context(tc.tile_pool(name="o", bufs=2))
        pspool = ctx.enter_context(tc.tile_pool(name="ps", bufs=2, space="PSUM"))

        # PE warmup: dummy matmuls on a memset tile so the HAM clock-gate
        # opens (~3.4us of sustained PE activity) and stays open while the
        # real loads land.  Rotating over all 4 psum tags keeps them dense.
        xwarm = wpool.tile([128, BT], f16, tag="warm", name="xwarm")
        nc.gpsimd.memset(xwarm[:], 0.0)
        for r in range(30):
            pw = pspool.tile([128, BT], f32, tag=f"pb{r % 4}", name=f"warm{r}")
            nc.tensor.matmul(pw[:], xwarm[:, :128], xwarm[:], start=True, stop=True)

        # eff^T = tanh(W^T) staged as [i_lane, i_chunk, o] f16.  The sync
        # HWDGE ring carries (in order) W chunk 0, x piece h0, W chunks 1-3,
        # x pieces h1-3, then whole quads — so the first matmul group's
        # inputs (eff chunk 0 via chunked tanh + x h0) land earliest.
        wti = wpool.tile([128, KI, BLOCK], f16, tag="w", name="wti")
        eff = wpool.tile([128, KI, BLOCK], f16, tag="e", name="eff")
        xq0 = xpool.tile([128, KI, QUAD], f16, tag="x", name="xq0")

        def load_w_chunk(i):
            nc.sync.dma_start(wti[:, i, :], wtv[:, i, :])
            nc.scalar.activation(
                eff[:, i, :], wti[:, i, :], mybir.ActivationFunctionType.Tanh
            )

        def load_x0_piece(h):
            nc.sync.dma_start(
                xq0[:, :, BT * h : BT * (h + 1)], xtv[:, :, BT * h : BT * (h + 1)]
            )

        load_w_chunk(0)
        load_x0_piece(0)
        for i in range(1, KI):
            load_w_chunk(i)
        for h in range(1, HT):
            load_x0_piece(h)

        for q in range(NQ):
            if q == 0:
                xq = xq0
            else:
                xq = xpool.tile([128, KI, QUAD], f16, tag="x", name=f"xq{q}")
                nc.sync.dma_start(xq[:], xtv[:, :, QUAD * q : QUAD * (q + 1)])

            for hh in range(HT // 2):
                stg = [
                    opool.tile(
                        [128, 2 * BT], f16, tag=f"o{o}h{hh}", name=f"st{o}_{q}_{hh}"
                    )
                    for o in range(OT)
                ]
                if q == 0:
                    # h-major: consume the arriving pieces in order
                    for hl in range(2):
                        h = 2 * hh + hl
                        for o in range(OT):
                            ps = pspool.tile(
                                [128, BT], f32, tag=f"pb{o}", name=f"ps{o}_{q}_{h}"
                            )
                            for i in range(KI):
                                nc.tensor.matmul(
                                    ps[:],
                                    eff[:, i, 128 * o : 128 * (o + 1)],
                                    xq[:, i, BT * h : BT * (h + 1)],
                                    start=(i == 0),
                                    stop=(i == KI - 1),
                                )
                            dst = stg[o][:, BT * hl : BT * (hl + 1)]
                            if o % 2 == 0:
                                nc.vector.tensor_copy(dst, ps[:])
                            else:
                                nc.scalar.copy(dst, ps[:])
                else:
                    # weight-reuse order: for each (o, i) the 4 consecutive
                    # matmuls share the stationary operand (one LDWEIGHTS),
                    # accumulating into 2 interleaved h-banks per half
                    for o in range(OT):
                        pss = [
                            pspool.tile(
                                [128, BT], f32, tag=f"pb{2 * (o % 2) + hl}",
                                name=f"ps{o}_{q}_{2 * hh + hl}",
                            )
                            for hl in range(2)
                        ]
                        for i in range(KI):
                            for hl in range(2):
                                h = 2 * hh + hl
                                nc.tensor.matmul(
                                    pss[hl][:],
                                    eff[:, i, 128 * o : 128 * (o + 1)],
                                    xq[:, i, BT * h : BT * (h + 1)],
                                    start=(i == 0),
                                    stop=(i == KI - 1),
                                )
                        for hl in range(2):
                            dst = stg[o][:, BT * hl : BT * (hl + 1)]
                            if o % 2 == 0:
                                nc.vector.tensor_copy(dst, pss[hl][:])
                            else:
                                nc.scalar.copy(dst, pss[hl][:])
                for o in range(OT):
                    # stores on the ACT HWDGE ring, disjoint from the load ring
                    nc.scalar.dma_start(
                        ot[
                            128 * o : 128 * (o + 1),
                            QUAD * q + 2 * BT * hh : QUAD * q + 2 * BT * (hh + 1),
                        ],
                        stg[o][:],
                    )

    nc.compile()
    return nc


def get_program():
    if "nc" not in _CACHED:
        _CACHED["nc"] = _build_program()
    return _CACHED["nc"]


def make_in_maps(x: np.ndarray, W: np.ndarray):
    xT16 = x.T.astype(np.float16)  # [N, BATCH] C-contiguous
    in_maps = []
    for k in range(NBLOCKS):
        sl = slice(BLOCK * k, BLOCK * (k + 1))
        in_maps.append(
            {
                "xt": np.ascontiguousarray(xT16[sl, :]),
                "wt": np.ascontiguousarray(W[sl, sl].T.astype(np.float16)),
            }
        )
    return in_maps


def assemble_output(results) -> np.ndarray:
    out = np.empty((BATCH, N), np.float32)
    for k in range(NBLOCKS):
        out[:, BLOCK * k : BLOCK * (k + 1)] = results[k]["ot"].T.astype(np.float32)
    return out


def kernel(x: np.ndarray, W: np.ndarray, mask: np.ndarray) -> np.ndarray:
    # mask is exactly the block-diagonal pattern (all-True inside each
    # diagonal 512 block); W is already zero off-block, so tanh(W*mask)
    # restricted to block k is tanh(W[blk_k, blk_k]).
    from concourse.bass_utils import run_bass_kernel_spmd

    nc = get_program()
    in_maps = make_in_maps(x, W)
    res = run_bass_kernel_spmd(nc, in_maps, list(range(NBLOCKS)))
    return assemble_output(res.results)
